# revision 1
# baseline (speedup 1.0000x reference)
"""GCN layer kernel for TRN2, data-parallel over batch across 8 NeuronCores.

Per core (one batch b):
  phase A: stream adjT (bf16 shadow) -> deg matvec on PE; load x, build xT via
           PE transposes.
  transition: deg -> dis -> u (col layout); z = u*x; c1/c2 row broadcast.
  phase B: agg0T[d,i] = sum_j adjT[j,i] * z[j,d] as fp32r matmuls, two half
           passes over i with 8 PSUM banks; epilogue folds the self loop:
           aggT = c1[i]*agg0T + c2[i]*xT.
  phase C: out2[l,o] = aggT.T @ W.T + b (bias via K=1 matmul), relu/scale,
           residual, layernorm via moments; stage-sliced emission (groups of
           4 row-blocks) to pipeline the strict-FIFO engines.
"""
import os
import numpy as np
import ml_dtypes

import concourse.bacc as bacc
import concourse.tile as tile
import concourse.mybir as mybir
from concourse.bass_utils import run_bass_kernel_spmd

B, L, D = 8, 2048, 512
JBN = L // 128      # 16 row blocks
NCH = L // 512      # 4 i-chunks of 512
DBN = D // 128      # 4 d-blocks
LN_EPS = 1e-5
DSCALE = float(D) ** -0.5
F32 = mybir.dt.float32
F32R = mybir.dt.float32r
BF16 = mybir.dt.bfloat16
MUL = mybir.AluOpType.mult
ADD = mybir.AluOpType.add
SUB = mybir.AluOpType.subtract

LAST_RESULT = None  # BassKernelResults of the most recent run (for profiling)


def _round_fp32r(v: np.ndarray) -> np.ndarray:
    """RNE-round fp32 to e8m11-in-top-20-bits (matches HW fp32r rounding)."""
    bits = np.ascontiguousarray(v, dtype=np.float32).view(np.uint32)
    r = bits + np.uint32(0x7FF) + ((bits >> np.uint32(12)) & np.uint32(1))
    r &= np.uint32(0xFFFFF000)
    return r.view(np.float32)


def _build_program(ln_identity=False, bias_zero=False):
    nc = bacc.Bacc("TRN2", target_bir_lowering=False, debug=False)
    d = {}
    def di(name, shape, dt):
        d[name] = nc.dram_tensor(name, shape, dt, kind="ExternalInput").ap()
    di("adjT_r", [L, L], F32R)
    di("adjT_h", [L, L], BF16)
    di("x_in", [L, D], F32)
    di("validc_f", [128, JBN], F32)
    di("validc_h", [128, JBN], BF16)
    di("ewc", [128, 1], F32)
    di("wt_r", [D, D], F32R)
    di("b_row_r", [1, D], F32R)
    di("ones_row", [1, 128], F32R)
    di("lnw_row", [1, D], F32)
    di("lnb_row", [1, D], F32)
    di("ident", [128, 128], F32)
    out_d = nc.dram_tensor("out_t", [L, D], F32, kind="ExternalOutput").ap()

    with tile.TileContext(nc) as tc:
        with tc.tile_pool(name="pX", bufs=JBN) as pX, \
             tc.tile_pool(name="pAgg", bufs=JBN) as pAgg, \
             tc.tile_pool(name="pW", bufs=DBN) as pW, \
             tc.tile_pool(name="pStat", bufs=1) as pStat, \
             tc.tile_pool(name="pCol", bufs=32) as pCol, \
             tc.tile_pool(name="pSmall", bufs=1) as pSmall:

            # ---- persistent arrays + global statics ----
            wt_t = [pW.tile([128, D], F32R, tag="wt", name=f"wt{k}")
                    for k in range(DBN)]
            eps_t = pSmall.tile([128, 1], F32, tag="eps")
            nc.vector.memset(eps_t[:], LN_EPS)
            ones_t = pSmall.tile([1, 128], F32R, tag="ones")
            nc.scalar.dma_start(ones_t[:], d["ones_row"][:])
            browr_t = pSmall.tile([1, D], F32R, tag="browr")
            nc.scalar.dma_start(browr_t[:], d["b_row_r"][:])
            x_t = [pX.tile([128, D], F32, tag="x", name=f"x{j}") for j in range(JBN)]
            agg_t = [pAgg.tile([128, D], F32R, tag="agg", name=f"agg{j}")
                     for j in range(JBN)]
            stat_b = {}

            with tc.tile_pool(name="pZ", bufs=JBN) as pZ, \
                 tc.tile_pool(name="pXT", bufs=DBN) as pXT, \
                 tc.tile_pool(name="pB", bufs=10) as pB, \
                 tc.tile_pool(name="pC", bufs=1) as pC, \
                 tc.tile_pool(name="psMM", bufs=4, space="PSUM") as psMM:
                psPT_cm = tc.tile_pool(name="psPT", bufs=2, space="PSUM")
                psPT = psPT_cm.__enter__()
                psMisc_cm = tc.tile_pool(name="psMisc", bufs=2, space="PSUM")
                psMisc = psMisc_cm.__enter__()
                z_t = [pZ.tile([128, D], F32R, tag="z", name=f"z{j}")
                       for j in range(JBN)]
                xT_t = [pXT.tile([128, L], BF16, tag="xT", name=f"xT{m}")
                        for m in range(DBN)]
                c1b = pC.tile([128, L], F32, tag="c1b")
                c2b = pC.tile([128, L], F32, tag="c2b")

                # ---- transient scope: phase A + transition ----
                with tc.tile_pool(name="pTrans", bufs=1) as pTrans, \
                     tc.tile_pool(name="pA", bufs=3) as pA:
                    ident_t = pTrans.tile([128, 128], F32, tag="ident")
                    nc.scalar.dma_start(ident_t[:], d["ident"][:])
                    validf_t = pTrans.tile([128, JBN], F32, tag="vf")
                    nc.scalar.dma_start(validf_t[:], d["validc_f"][:])
                    validh_t = pTrans.tile([128, JBN], BF16, tag="vh")
                    nc.scalar.dma_start(validh_t[:], d["validc_h"][:])
                    ewc_t = pTrans.tile([128, 1], F32, tag="ew")
                    nc.scalar.dma_start(ewc_t[:], d["ewc"][:])
                    rows = {}
                    for nm in ("lnw_row", "lnb_row"):
                        r = pTrans.tile([1, D], F32, tag=nm, name=nm + "_t")
                        nc.scalar.dma_start(r[:], d[nm][:])
                        rows[nm] = r
                    for nm in ("lnw_row", "lnb_row"):
                        t = pStat.tile([128, D], F32, tag=nm + "b", name=nm + "_b")
                        nc.gpsimd.partition_broadcast(t[:], rows[nm][:])
                        stat_b[nm] = t

                    # phase A: deg matvecs (bf16, N=1, col layout) + x load
                    # + xT build on PE
                    deg_ps = [psMisc.tile([128, 512], F32, tag="misc",
                                          name=f"deg_ps{i}") for i in range(2)]
                    for jb in range(JBN):
                        adjA = pA.tile([128, L], BF16, tag="adjA")
                        nc.sync.dma_start(
                            adjA[:], d["adjT_h"][jb * 128:(jb + 1) * 128, :])
                        for n in range(NCH):
                            po = 32 * (n % 2)
                            nc.tensor.matmul(
                                deg_ps[n // 2][po:po + 1, :],
                                validh_t[:, jb:jb + 1],
                                adjA[:, n * 512:(n + 1) * 512],
                                start=(jb == 0), stop=(jb == JBN - 1))
                        nc.scalar.dma_start(
                            x_t[jb][:], d["x_in"][jb * 128:(jb + 1) * 128, :])
                        for m in range(DBN):
                            pt = psPT.tile([128, 128], F32, tag="pt")
                            nc.tensor.transpose(
                                pt[:], x_t[jb][:, m * 128:(m + 1) * 128],
                                ident_t[:])
                            nc.vector.tensor_copy(
                                xT_t[m][:, jb * 128:(jb + 1) * 128], pt[:])
                    r_sb = pTrans.tile([128, 1024], F32, tag="rsb")
                    for n in range(NCH):
                        po = 32 * (n % 2)
                        nc.vector.tensor_copy(
                            r_sb[po:po + 1, (n // 2) * 512:(n // 2 + 1) * 512],
                            deg_ps[n // 2][po:po + 1, :])
                    rc_ps = psMisc.tile([128, JBN], F32, tag="misc", name="rc_ps")
                    for v in range(JBN):
                        n, c = v // 4, v % 4
                        po = 32 * (n % 2)
                        fo = (n // 2) * 512 + c * 128
                        nc.tensor.transpose(
                            rc_ps[:, v:v + 1],
                            r_sb[po:po + 1, fo:fo + 128],
                            ident_t[po:po + 1, po:po + 1])
                    r_col = pCol.tile([128, JBN], F32, tag="rcol", bufs=1)
                    nc.vector.tensor_copy(r_col[:], rc_ps[:])

                    deg_col = pCol.tile([128, JBN], F32, tag="degc", bufs=1)
                    nc.vector.tensor_mul(deg_col[:], r_col[:], validf_t[:])
                    nc.vector.tensor_scalar_add(deg_col[:], deg_col[:], 1.0)
                    std_col = pCol.tile([128, JBN], F32, tag="stdc", bufs=1)
                    nc.scalar.sqrt(std_col[:], deg_col[:])
                    dis_col = pCol.tile([128, JBN], F32, tag="disc", bufs=1)
                    nc.vector.reciprocal(dis_col[:], std_col[:])
                    u_col = pCol.tile([128, JBN], F32, tag="uc", bufs=1)
                    nc.vector.tensor_mul(u_col[:], dis_col[:], validf_t[:])

                    c1_col = pCol.tile([128, JBN], F32, tag="c1c", bufs=1)
                    nc.vector.tensor_scalar_mul(c1_col[:], u_col[:], ewc_t[:])
                    c2_col = pCol.tile([128, JBN], F32, tag="c2c", bufs=1)
                    nc.vector.scalar_tensor_tensor(
                        c2_col[:], dis_col[:], ewc_t[:], dis_col[:], MUL, MUL)

                    # c1/c2 -> row chunks -> one partition_broadcast per vector
                    for nm, col, bc in (("c1", c1_col, c1b), ("c2", c2_col, c2b)):
                        rcf = pTrans.tile([1, L], F32, tag="crow", bufs=1,
                                          name=f"{nm}rowf")
                        for n in range(NCH):
                            rp = psMisc.tile([1, 512], F32, tag="misc",
                                             name=f"{nm}rp{n}")
                            for q in range(4):
                                v = n * 4 + q
                                nc.tensor.transpose(
                                    rp[0:1, q * 128:(q + 1) * 128],
                                    col[:, v:v + 1], ident_t[:])
                            nc.vector.tensor_copy(rcf[:, n * 512:(n + 1) * 512],
                                                  rp[:])
                        nc.gpsimd.partition_broadcast(bc[:], rcf[:])

                    # z tiles (DVE rounds to fp32r)
                    for jb in range(JBN):
                        nc.vector.tensor_scalar_mul(z_t[jb][:], x_t[jb][:],
                                                    u_col[:, jb:jb + 1])

                # ---- close phase-A psum pools; open C-side pools ----
                psMisc_cm.__exit__(None, None, None)
                psPT_cm.__exit__(None, None, None)

                for k in range(DBN):
                    nc.scalar.dma_start(wt_t[k][:],
                                        d["wt_r"][k * 128:(k + 1) * 128, :])

                # ---- fused phases B & C: pass p feeds layernorm group p ----
                G = 4
                with tc.tile_pool(name="pScr", bufs=16) as pScr, \
                     tc.tile_pool(name="pOut", bufs=5) as pOut, \
                     tc.tile_pool(name="psC", bufs=4, space="PSUM") as psC:
                    mm_ps = {}
                    for p in range(NCH):
                        # -- pass p: MM1 quarter
                        for m in range(DBN):
                            mm_ps[(p, m)] = psMM.tile([128, 512], F32, tag="mm",
                                                      name=f"mm1_{p}_{m}")
                        for jb in range(JBN):
                            jsl = slice(jb * 128, (jb + 1) * 128)
                            adjQ = pB.tile([128, 512], F32R, tag="adjB")
                            nc.sync.dma_start(
                                adjQ[:], d["adjT_r"][jsl, p * 512:(p + 1) * 512])
                            for m in range(DBN):
                                nc.tensor.matmul(
                                    mm_ps[(p, m)][:],
                                    z_t[jb][:, m * 128:(m + 1) * 128],
                                    adjQ[:], start=(jb == 0), stop=(jb == JBN - 1))
                        # -- epilogue p: aggT = c1*agg0T + c2*xT
                        sl = slice(p * 512, (p + 1) * 512)
                        t2d = {}
                        for m in range(DBN):
                            t2 = pScr.tile([128, 512], F32, tag="scr",
                                           name=f"t2_{p}_{m}")
                            nc.vector.tensor_mul(t2[:], mm_ps[(p, m)][:],
                                                 c1b[:, sl])
                            t2d[m] = t2
                        for m in range(DBN):
                            tmp = pScr.tile([128, 512], F32, tag="scr",
                                            name=f"tp_{p}_{m}")
                            nc.gpsimd.tensor_mul(tmp[:], xT_t[m][:, sl],
                                                 c2b[:, sl])
                            nc.gpsimd.tensor_add(agg_t[m * NCH + p][:],
                                                 t2d[m][:], tmp[:])
                        # -- layernorm group p: lbs 4p..4p+3
                        lbs = list(range(G * p, G * (p + 1)))
                        ps2d, rd, hhd, sumd, m2d = {}, {}, {}, {}, {}
                        mud, rstdd, t1d = {}, {}, {}
                        for lb in lbs:
                            n, off = lb // 4, (lb % 4) * 128
                            ps2 = psC.tile([128, D], F32, tag="mmc",
                                           name=f"mm2_{lb}")
                            for k in range(DBN):
                                nc.tensor.matmul(
                                    ps2[:], agg_t[k * NCH + n][:, off:off + 128],
                                    wt_t[k][:], start=(k == 0),
                                    stop=(bias_zero and k == DBN - 1))
                            if not bias_zero:
                                nc.tensor.matmul(ps2[:], ones_t[:], browr_t[:],
                                                 start=False, stop=True)
                            ps2d[lb] = ps2
                        for lb in lbs:
                            r = pScr.tile([128, D], F32, tag="scr", name=f"r{lb}")
                            nc.scalar.activation(r[:], ps2d[lb][:],
                                                 mybir.ActivationFunctionType.Relu,
                                                 scale=DSCALE)
                            rd[lb] = r
                        for lb in lbs:
                            hh = pScr.tile([128, D], F32, tag="scr", name=f"hh{lb}")
                            sums = pCol.tile([128, 1], F32, tag="lncol",
                                             name=f"su{lb}")
                            nc.vector.scalar_tensor_tensor(
                                hh[:], rd[lb][:], 1.0, x_t[lb][:], MUL, ADD,
                                accum_out=sums[:])
                            hhd[lb], sumd[lb] = hh, sums
                        for lb in lbs:
                            sq = pScr.tile([128, D], F32, tag="scr", name=f"sq{lb}")
                            m2s = pCol.tile([128, 1], F32, tag="lncol",
                                            name=f"m2{lb}")
                            nc.vector.scalar_tensor_tensor(
                                sq[:], hhd[lb][:], 1.0, hhd[lb][:], MUL, MUL,
                                accum_out=m2s[:])
                            m2d[lb] = m2s
                        for lb in lbs:
                            mu = pCol.tile([128, 1], F32, tag="lncol",
                                           name=f"mu{lb}")
                            nc.scalar.mul(mu[:], sumd[lb][:], 1.0 / D)
                            m2n = pCol.tile([128, 1], F32, tag="lncol",
                                            name=f"mn{lb}")
                            nc.scalar.mul(m2n[:], m2d[lb][:], 1.0 / D)
                            negv = pCol.tile([128, 1], F32, tag="lncol",
                                             name=f"nv{lb}")
                            nc.vector.scalar_tensor_tensor(
                                negv[:], mu[:], mu[:], m2n[:], MUL, SUB)
                            stdt = pCol.tile([128, 1], F32, tag="lncol",
                                             name=f"sd{lb}")
                            nc.scalar.activation(
                                stdt[:], negv[:],
                                mybir.ActivationFunctionType.Sqrt,
                                scale=-1.0, bias=eps_t[:])
                            rstd = pCol.tile([128, 1], F32, tag="lncol",
                                             name=f"rs{lb}")
                            nc.vector.reciprocal(rstd[:], stdt[:])
                            mud[lb], rstdd[lb] = mu, rstd
                        for lb in lbs:
                            eng1 = nc.gpsimd if lb % 2 == 0 else nc.vector
                            t1 = pOut.tile([128, D], F32, tag="o", name=f"t1{lb}")
                            eng1.tensor_scalar(t1[:], hhd[lb][:], mud[lb][:],
                                               rstdd[lb][:], SUB, MUL)
                            t1d[lb] = t1
                        if ln_identity:
                            for lb in lbs:
                                nc.sync.dma_start(
                                    out_d[lb * 128:(lb + 1) * 128, :], t1d[lb][:])
                        else:
                            for lb in lbs:
                                tt = pScr.tile([128, D], F32, tag="scr",
                                               name=f"tt{lb}")
                                teng = nc.vector if lb % 2 == 0 else nc.gpsimd
                                teng.tensor_mul(tt[:], t1d[lb][:],
                                                stat_b["lnw_row"][:])
                                o_sb = pOut.tile([128, D], F32, tag="o",
                                                 name=f"o{lb}")
                                nc.gpsimd.tensor_add(o_sb[:], tt[:],
                                                     stat_b["lnb_row"][:])
                                nc.sync.dma_start(
                                    out_d[lb * 128:(lb + 1) * 128, :], o_sb[:])

    nc.compile()
    return nc


_NC_CACHE = {}


def _get_nc(ln_identity=False, bias_zero=False):
    key = (ln_identity, bias_zero)
    if key not in _NC_CACHE:
        _NC_CACHE[key] = _build_program(*key)
    return _NC_CACHE[key]


def kernel(x, adj, pad_mask, W, b, ln_w, ln_b, edge_weight):
    global LAST_RESULT
    x = np.asarray(x, dtype=np.float32)
    adj = np.asarray(adj, dtype=np.float32)
    pad_mask = np.asarray(pad_mask)
    W = np.asarray(W, dtype=np.float32)
    b = np.asarray(b, dtype=np.float32)
    ln_w = np.asarray(ln_w, dtype=np.float32)
    ln_b = np.asarray(ln_b, dtype=np.float32)
    ew = float(np.asarray(edge_weight).reshape(-1)[0])

    ln_identity = bool(np.all(ln_w == 1.0) and np.all(ln_b == 0.0))
    bias_zero = bool(np.all(b == 0.0))
    nc = _get_nc(ln_identity, bias_zero)

    wt_r = _round_fp32r(np.ascontiguousarray(W.T))
    ewc = np.full((128, 1), ew, dtype=np.float32)
    ident = np.eye(128, dtype=np.float32)
    b_row_r = _round_fp32r(b.reshape(1, D))
    ones_row = np.ones((1, 128), dtype=np.float32)
    lnw_row = np.ascontiguousarray(ln_w.reshape(1, D))
    lnb_row = np.ascontiguousarray(ln_b.reshape(1, D))

    in_maps = []
    for c in range(B):
        adjT = np.ascontiguousarray(adj[c].T)
        valid = (~pad_mask[c]).astype(np.float32)
        validc = np.ascontiguousarray(valid.reshape(JBN, 128).T)
        in_maps.append({
            "adjT_r": _round_fp32r(adjT),
            "adjT_h": adjT.astype(ml_dtypes.bfloat16),
            "x_in": np.ascontiguousarray(x[c]),
            "validc_f": validc,
            "validc_h": validc.astype(ml_dtypes.bfloat16),
            "ewc": ewc,
            "wt_r": wt_r,
            "b_row_r": b_row_r,
            "ones_row": ones_row,
            "lnw_row": lnw_row,
            "lnb_row": lnb_row,
            "ident": ident,
        })

    trace = os.environ.get("KERNEL_TRACE", "0") == "1"
    res = run_bass_kernel_spmd(nc, in_maps, core_ids=list(range(B)), trace=trace)
    LAST_RESULT = res
    out = np.stack([res.results[c]["out_t"] for c in range(B)], axis=0)
    return out

